# revision 9
# baseline (speedup 1.0000x reference)
"""GQA attention (B=2, T=2048, D=4096, H=32, G=8, d=128) on 8 TRN2 NeuronCores.

Sharding: one KV group per core (4 Q heads + 1 K/V head). All-bf16 matmuls
(fp8 measured over the error budget on this data: softmax rows are near
one-hot, so 8-bit probs/values lose ~1.5e-2 rel).

Single woven instruction stream: projection jobs, attention head-jobs and
out-projection Do-blocks are interleaved at ~7us granularity so the ACT-bound
exp stretches and DVE mask/normalize work always overlap PE-bound GEMMs.
Causal mask is applied by DVE on PSUM (saves PE mask matmuls); diagonal score
matmuls shrink to the unmasked range (mask covers the stale PSUM region).
Partial outputs are written fp16 and summed on host.
"""

import math
import sys
from collections import deque

import numpy as np

sys.path.insert(0, "/opt/trn_rl_repo")

import ml_dtypes

BF16 = ml_dtypes.bfloat16

B, T, D = 2, 2048, 4096
H, G, d = 32, 8, 128
GROUP = H // G  # 4
NT = B * T
NC_ = 8

TOK = 512
DC = D // 128  # 32
HDC = DC // 2  # 16
KTB = T // 128  # 16

_program_cache = {}


def _build_program():
    import concourse.mybir as mybir
    import concourse.tile as tile
    from concourse import bacc
    from concourse.bass import ds, ts
    from concourse.masks import make_identity

    f32 = mybir.dt.float32
    bf16 = mybir.dt.bfloat16
    f16 = mybir.dt.float16
    AF = mybir.ActivationFunctionType

    nc = bacc.Bacc()

    xt_d = nc.declare_dram_parameter("xt", [128, DC, NT], bf16, isOutput=False)
    wq_d = nc.declare_dram_parameter("wq", [128, DC * GROUP, 128], bf16, isOutput=False)
    wk_d = nc.declare_dram_parameter("wk", [128, DC, 128], bf16, isOutput=False)
    wv_d = nc.declare_dram_parameter("wv", [128, DC, 128], bf16, isOutput=False)
    wo_d = nc.declare_dram_parameter("wo", [128, GROUP * DC, 128], bf16, isOutput=False)
    bq_d = nc.declare_dram_parameter("bq", [128, GROUP], f32, isOutput=False)
    bk_d = nc.declare_dram_parameter("bk", [128, 1], f32, isOutput=False)
    bv_d = nc.declare_dram_parameter("bv", [128, 1], f32, isOutput=False)
    mask_d = nc.declare_dram_parameter("masks", [128, 4, TOK], bf16, isOutput=False)
    ones_d = nc.declare_dram_parameter("ones128", [128, 128], bf16, isOutput=False)
    out_d = nc.declare_dram_parameter("out", [128, DC, NT], f16, isOutput=True)

    with tile.TileContext(nc) as tc:
        with (
            tc.tile_pool(name="persist", bufs=1) as persist,
            tc.tile_pool(name="xp", bufs=2) as xp,
            tc.tile_pool(name="qy", bufs=3) as qyp,
            tc.tile_pool(name="pp", bufs=3) as ppl,
            tc.tile_pool(name="misc", bufs=2) as mis,
            tc.psum_pool(name="gemm", bufs=3) as gps,
            tc.psum_pool(name="st", bufs=2) as stp,
            tc.psum_pool(name="yps", bufs=2) as ypp,
            tc.psum_pool(name="lps", bufs=1) as lpp,
        ):
            kT = persist.tile([128, NT], bf16)
            vtm = persist.tile([128, NT // 128, 128], bf16)
            maskb = persist.tile([128, 4, TOK], bf16)
            bq_s = persist.tile([128, GROUP], f32)
            bk_s = persist.tile([128, 1], f32)
            bv_s = persist.tile([128, 1], f32)
            ones128 = persist.tile([128, 128], bf16)
            ident = persist.tile([128, 128], bf16)
            wq_s = persist.tile([128, GROUP * DC, 128], bf16)
            wk_s = persist.tile([128, DC, 128], bf16)
            wv_s = persist.tile([128, DC, 128], bf16)
            wo_s = persist.tile([128, GROUP * DC, 128], bf16)

            # first-needed first: the opening K-projection reads wk chunk 0
            # and the first quarter of xa0; spread across engine DGE queues so
            # the transfers run in parallel
            nc.scalar.dma_start(out=wk_s[:, ds(0, 8), :], in_=wk_d[:, ds(0, 8), :])

            def dma_x(tt):
                xa = xp.tile([128, HDC, TOK], bf16, tag="xa", name=f"xa{tt}")
                xb = xp.tile([128, HDC, TOK], bf16, tag="xb", name=f"xb{tt}")
                QD = HDC // 4
                for qd in range(4):
                    nc.sync.dma_start(
                        out=xa[:, ts(qd, QD), :],
                        in_=xt_d[:, ds(qd * QD, QD), ts(tt, TOK)],
                    )
                for qd in range(4):
                    nc.sync.dma_start(
                        out=xb[:, ts(qd, QD), :],
                        in_=xt_d[:, ds(HDC + qd * QD, QD), ts(tt, TOK)],
                    )
                return xa, xb

            xab0 = dma_x(0)
            for qd in range(1, 4):
                nc.scalar.dma_start(
                    out=wk_s[:, ds(qd * 8, 8), :], in_=wk_d[:, ds(qd * 8, 8), :]
                )
            nc.scalar.dma_start(out=bk_s[:], in_=bk_d[:])
            nc.scalar.dma_start(out=bv_s[:], in_=bv_d[:])
            nc.scalar.dma_start(out=bq_s[:], in_=bq_d[:])
            nc.scalar.dma_start(out=wv_s[:], in_=wv_d[:])
            nc.sync.dma_start(out=maskb[:], in_=mask_d[:])
            nc.sync.dma_start(out=ones128[:], in_=ones_d[:])
            make_identity(nc, ident[:])
            for dq in range(GROUP):
                nc.scalar.dma_start(
                    out=wq_s[:, ts(dq, DC), :], in_=wq_d[:, ts(dq, DC), :]
                )
            nc.sync.dma_start(out=wo_s[:], in_=wo_d[:])

            # ---- job bodies ----

            def proj_kv(tt, xa, xb):
                b = tt // 4

                def xsrc(Dc):
                    return (xa if Dc < HDC else xb)[:, Dc % HDC, :]

                ps = gps.tile([128, TOK], f32, tag="gemm", name="psk")
                for Dc in range(DC):
                    nc.tensor.matmul(
                        ps[:], lhsT=wk_s[:, Dc, :], rhs=xsrc(Dc),
                        start=(Dc == 0), stop=(Dc == DC - 1),
                    )
                nc.scalar.activation(
                    out=kT[:, ts(tt, TOK)], in_=ps[:], func=AF.Identity,
                    bias=bk_s[:, 0:1],
                )
                ps = gps.tile([128, TOK], f32, tag="gemm", name="psv")
                for Dc in range(DC):
                    nc.tensor.matmul(
                        ps[:], lhsT=wv_s[:, Dc, :], rhs=xsrc(Dc),
                        start=(Dc == 0), stop=(Dc == DC - 1),
                    )
                vT = mis.tile([128, TOK], bf16, tag="vt")
                nc.scalar.activation(
                    out=vT[:], in_=ps[:], func=AF.Identity, bias=bv_s[:, 0:1]
                )
                for j in range(4):
                    pt = gps.tile([128, 128], bf16, tag="gemm", name="pt")
                    nc.tensor.transpose(pt[:], vT[:, ts(j, 128)], ident[:])
                    nc.vector.tensor_copy(
                        out=vtm[:, b * KTB + (tt % 4) * 4 + j, :], in_=pt[:]
                    )

            def proj_q(tt, xa, xb, qt, dq):
                def xsrc(Dc):
                    return (xa if Dc < HDC else xb)[:, Dc % HDC, :]

                ps = gps.tile([128, TOK], f32, tag="gemm", name="psq")
                for Dc in range(DC):
                    nc.tensor.matmul(
                        ps[:], lhsT=wq_s[:, dq * DC + Dc, :], rhs=xsrc(Dc),
                        start=(Dc == 0), stop=(Dc == DC - 1),
                    )
                nc.scalar.activation(
                    out=qt[:, dq, :], in_=ps[:], func=AF.Identity,
                    bias=bq_s[:, dq : dq + 1],
                )

            def attn_head(tt, qt, yt, h, filler):
                qi, b = tt % 4, tt // 4
                njt = 4 * (qi + 1)
                yps = ypp.tile([128, TOK], f32, tag="yps")
                lps = lpp.tile([128, TOK], f32, tag="lps")
                for jp in range(njt // 2):
                    p16 = ppl.tile([128, 2, TOK], bf16, tag="p16", bufs=5)
                    for jj in range(2):
                        j = jp * 2 + jj
                        r = j - 4 * qi
                        st = stp.tile([128, TOK], f32, tag="st")
                        lo = 0 if r <= 0 else r * 128
                        nc.tensor.matmul(
                            st[:, lo:TOK],
                            lhsT=kT[:, ds(b * T + j * 128, 128)],
                            rhs=qt[:, h, lo:TOK],
                            start=True, stop=True,
                        )
                        if r >= 0:
                            nc.vector.tensor_add(
                                out=st[:], in0=st[:], in1=maskb[:, r, :]
                            )
                        nc.scalar.activation(
                            out=p16[:, jj, :], in_=st[:], func=AF.Exp,
                        )
                    for jj in range(2):
                        j = jp * 2 + jj
                        nc.tensor.matmul(
                            lps[:], lhsT=ones128[:], rhs=p16[:, jj, :],
                            start=(j == 0), stop=(j == njt - 1),
                        )
                        nc.tensor.matmul(
                            yps[:], lhsT=vtm[:, b * KTB + j, :],
                            rhs=p16[:, jj, :],
                            start=(j == 0), stop=(j == njt - 1),
                        )
                    if jp < njt // 2 - 1:
                        filler()
                inv = mis.tile([128, TOK], f32, tag="inv", bufs=1)
                nc.vector.reciprocal_approx_fast(out=inv[:], in_=lps[:])
                nc.vector.tensor_mul(out=yt[:, h, :], in0=yps[:], in1=inv[:])
                filler()

            def oproj_chunk(tt, yt, Do):
                ps = gps.tile([128, TOK], f32, tag="gemm", name="pso")
                for c in range(GROUP):
                    nc.tensor.matmul(
                        ps[:], lhsT=wo_s[:, c * DC + Do, :], rhs=yt[:, c, :],
                        start=(c == 0), stop=(c == GROUP - 1),
                    )
                so = mis.tile([128, TOK], f16, tag="so", bufs=3)
                nc.vector.tensor_copy(out=so[:], in_=ps[:])
                nc.sync.dma_start(out=out_d[:, Do, ts(tt, TOK)], in_=so[:])

            # ---- woven schedule ----
            # tt order: batches interleaved, causality respected.
            tt_order = [0, 4, 1, 5, 2, 6, 3, 7]
            heads = deque()  # pending attention-head jobs
            fillq = deque()  # pending oproj Do-chunk jobs (PE filler work)

            def filler():
                if fillq:
                    fillq.popleft()()

            def pump(n):
                for _ in range(n):
                    if heads:
                        heads.popleft()()
                    elif fillq:
                        fillq.popleft()()
                    else:
                        return

            def head_job(tt, h):
                attn_head(tt, qts[tt], yts[tt], h, filler)
                if h == GROUP - 1:
                    # all heads of tt emitted: its out-proj chunks are now
                    # safe to emit (yt fully written in program order)
                    for Do in range(DC):
                        fillq.append(
                            lambda tt=tt, Do=Do: oproj_chunk(tt, yts[tt], Do)
                        )

            xabs = {0: xab0}
            qts, yts = {}, {}
            for s, tt in enumerate(tt_order):
                if tt not in xabs:
                    xabs[tt] = dma_x(tt)
                xa, xb = xabs[tt]
                nxt = tt_order[s + 1] if s + 1 < len(tt_order) else None
                proj_kv(tt, xa, xb)
                if nxt is not None:
                    xabs[nxt] = dma_x(nxt)
                qt = qyp.tile([128, GROUP, TOK], bf16, tag="qt", bufs=2, name=f"qt{tt}")
                yt = qyp.tile([128, GROUP, TOK], bf16, tag="yt", bufs=3, name=f"yt{tt}")
                qts[tt], yts[tt] = qt, yt
                for dq in range(GROUP):
                    proj_q(tt, xa, xb, qt, dq)
                    pump(1)  # one head job
                    for _ in range(3):
                        filler()  # a few oproj chunks
                prev = tt
                for h in range(GROUP):
                    heads.append(lambda tt=prev, h=h: head_job(tt, h))
                for _ in range(4):
                    filler()
            # drain
            while heads or fillq:
                pump(1)
                for _ in range(3):
                    filler()

    if not nc.is_finalized():
        nc.finalize()
    return nc


def _prep_inputs(hidden_states, Wq, bq, Wk, bk, Wv, bv, Wo, bo):
    scale = 1.0 / math.sqrt(d)

    x_flat = np.asarray(hidden_states, dtype=np.float32).reshape(NT, D)
    xt = np.ascontiguousarray(
        x_flat.reshape(NT, DC, 128).transpose(2, 1, 0)
    ).astype(BF16)

    jj = np.arange(128)[:, None, None]
    rr = np.arange(4)[None, :, None] * 128
    ii = np.arange(TOK)[None, None, :]
    masks = np.where(jj + rr > ii, np.float32(-1e9), np.float32(0.0)).astype(BF16)
    masks = np.ascontiguousarray(masks)

    ones128 = np.ones((128, 128), dtype=BF16)

    in_maps = []
    for g in range(NC_):
        Wq_g = np.asarray(Wq[g * 512 : (g + 1) * 512, :], dtype=np.float32) * scale
        bq_g = np.asarray(bq[g * 512 : (g + 1) * 512], dtype=np.float32) * scale
        Wk_g = np.asarray(Wk[g * 128 : (g + 1) * 128, :], dtype=np.float32)
        bk_g = np.asarray(bk[g * 128 : (g + 1) * 128], dtype=np.float32)
        Wv_g = np.asarray(Wv[g * 128 : (g + 1) * 128], dtype=np.float32)
        bv_g = np.asarray(bv[g * 128 : (g + 1) * 128], dtype=np.float32)
        Wo_g = np.asarray(Wo[:, g * 512 : (g + 1) * 512], dtype=np.float32)

        wq_host = np.ascontiguousarray(
            Wq_g.reshape(GROUP, 128, DC, 128).transpose(3, 0, 2, 1).reshape(
                128, GROUP * DC, 128
            )
        ).astype(BF16)
        wk_host = np.ascontiguousarray(
            Wk_g.reshape(128, DC, 128).transpose(2, 1, 0)
        ).astype(BF16)
        wv_host = np.ascontiguousarray(
            Wv_g.reshape(128, DC, 128).transpose(2, 1, 0)
        ).astype(BF16)
        wo_host = np.ascontiguousarray(
            Wo_g.reshape(DC, 128, GROUP, 128).transpose(3, 2, 0, 1).reshape(
                128, GROUP * DC, 128
            )
        ).astype(BF16)

        in_maps.append(
            {
                "xt": xt,
                "wq": wq_host,
                "wk": wk_host,
                "wv": wv_host,
                "wo": wo_host,
                "bq": np.ascontiguousarray(bq_g.reshape(GROUP, 128).T),
                "bk": bk_g.reshape(128, 1).copy(),
                "bv": bv_g.reshape(128, 1).copy(),
                "masks": masks,
                "ones128": ones128,
            }
        )
    return in_maps


def kernel(
    hidden_states, Wq, bq, Wk, bk, Wv, bv, Wo, bo, _trace=False, _result_box=None
):
    from concourse.bass_utils import run_bass_kernel_spmd

    if "nc" not in _program_cache:
        _program_cache["nc"] = _build_program()
    nc = _program_cache["nc"]

    in_maps = _prep_inputs(hidden_states, Wq, bq, Wk, bk, Wv, bv, Wo, bo)
    res = run_bass_kernel_spmd(
        nc, in_maps, core_ids=list(range(NC_)), trace=_trace
    )
    if _result_box is not None:
        _result_box.append(res)

    acc = np.zeros((128, DC, NT), dtype=np.float32)
    for r in res.results:
        acc += r["out"].astype(np.float32)
    outT = acc.transpose(1, 0, 2).reshape(D, NT)
    out = outT.T + np.asarray(bo, dtype=np.float32)[None, :]
    return np.ascontiguousarray(out.reshape(B, T, D), dtype=np.float32)


# revision 10
# speedup vs baseline: 1.1965x; 1.1965x over previous
"""GQA attention (B=2, T=2048, D=4096, H=32, G=8, d=128) on 8 TRN2 NeuronCores.

Sharding: one KV group per core (4 Q heads + 1 K/V head). All-bf16 matmuls
(fp8 measured over the error budget on this data: softmax rows are near
one-hot, so 8-bit probs/values lose ~1.5e-2 rel).

Single woven instruction stream: projection jobs, attention head-jobs and
out-projection Do-blocks are interleaved at ~7us granularity so the ACT-bound
exp stretches and DVE mask/normalize work always overlap PE-bound GEMMs.
Causal mask is applied by DVE on PSUM (saves PE mask matmuls); diagonal score
matmuls shrink to the unmasked range (mask covers the stale PSUM region).
Partial outputs are written fp16 and summed on host.
"""

import math
import sys
from collections import deque

import numpy as np

sys.path.insert(0, "/opt/trn_rl_repo")

import ml_dtypes

BF16 = ml_dtypes.bfloat16

B, T, D = 2, 2048, 4096
H, G, d = 32, 8, 128
GROUP = H // G  # 4
NT = B * T
NC_ = 8

TOK = 512
DC = D // 128  # 32
HDC = DC // 2  # 16
KTB = T // 128  # 16

_program_cache = {}


def _build_program():
    import concourse.mybir as mybir
    import concourse.tile as tile
    from concourse import bacc
    from concourse.bass import ds, ts
    from concourse.masks import make_identity

    f32 = mybir.dt.float32
    bf16 = mybir.dt.bfloat16
    f16 = mybir.dt.float16
    AF = mybir.ActivationFunctionType

    nc = bacc.Bacc()

    xt_d = nc.declare_dram_parameter("xt", [128, DC, NT], bf16, isOutput=False)
    wq_d = nc.declare_dram_parameter("wq", [128, DC * GROUP, 128], bf16, isOutput=False)
    wk_d = nc.declare_dram_parameter("wk", [128, DC, 128], bf16, isOutput=False)
    wv_d = nc.declare_dram_parameter("wv", [128, DC, 128], bf16, isOutput=False)
    wo_d = nc.declare_dram_parameter("wo", [128, GROUP * DC, 128], bf16, isOutput=False)
    bq_d = nc.declare_dram_parameter("bq", [128, GROUP], f32, isOutput=False)
    bk_d = nc.declare_dram_parameter("bk", [128, 1], f32, isOutput=False)
    bv_d = nc.declare_dram_parameter("bv", [128, 1], f32, isOutput=False)
    mask_d = nc.declare_dram_parameter("masks", [128, 4, TOK], bf16, isOutput=False)
    ones_d = nc.declare_dram_parameter("ones128", [128, 128], bf16, isOutput=False)
    out_d = nc.declare_dram_parameter("out", [128, DC, NT], f16, isOutput=True)

    with tile.TileContext(nc) as tc:
        with (
            tc.tile_pool(name="persist", bufs=1) as persist,
            tc.tile_pool(name="xp", bufs=2) as xp,
            tc.tile_pool(name="qy", bufs=3) as qyp,
            tc.tile_pool(name="pp", bufs=3) as ppl,
            tc.tile_pool(name="misc", bufs=2) as mis,
            tc.psum_pool(name="gemm", bufs=3) as gps,
            tc.psum_pool(name="st", bufs=2) as stp,
            tc.psum_pool(name="yps", bufs=2) as ypp,
            tc.psum_pool(name="lps", bufs=1) as lpp,
        ):
            kT = persist.tile([128, NT], bf16)
            vtm = persist.tile([128, NT // 128, 128], bf16)
            maskb = persist.tile([128, 4, TOK], bf16)
            bq_s = persist.tile([128, GROUP], f32)
            bk_s = persist.tile([128, 1], f32)
            bv_s = persist.tile([128, 1], f32)
            ones128 = persist.tile([128, 128], bf16)
            ident = persist.tile([128, 128], bf16)
            wq_s = persist.tile([128, GROUP * DC, 128], bf16)
            wk_s = persist.tile([128, DC, 128], bf16)
            wv_s = persist.tile([128, DC, 128], bf16)
            wo_s = persist.tile([128, GROUP * DC, 128], bf16)

            # first-needed first: the opening K-projection reads wk chunk 0
            # and the first quarter of xa0; everything else can trail
            nc.sync.dma_start(out=wk_s[:, ds(0, 8), :], in_=wk_d[:, ds(0, 8), :])

            def dma_x(tt):
                xa = xp.tile([128, HDC, TOK], bf16, tag="xa", name=f"xa{tt}")
                xb = xp.tile([128, HDC, TOK], bf16, tag="xb", name=f"xb{tt}")
                QD = HDC // 4
                for qd in range(4):
                    nc.sync.dma_start(
                        out=xa[:, ts(qd, QD), :],
                        in_=xt_d[:, ds(qd * QD, QD), ts(tt, TOK)],
                    )
                for qd in range(4):
                    nc.sync.dma_start(
                        out=xb[:, ts(qd, QD), :],
                        in_=xt_d[:, ds(HDC + qd * QD, QD), ts(tt, TOK)],
                    )
                return xa, xb

            xab0 = dma_x(0)
            for qd in range(1, 4):
                nc.sync.dma_start(
                    out=wk_s[:, ds(qd * 8, 8), :], in_=wk_d[:, ds(qd * 8, 8), :]
                )
            nc.sync.dma_start(out=bk_s[:], in_=bk_d[:])
            nc.sync.dma_start(out=bv_s[:], in_=bv_d[:])
            nc.sync.dma_start(out=bq_s[:], in_=bq_d[:])
            nc.sync.dma_start(out=wv_s[:], in_=wv_d[:])
            nc.sync.dma_start(out=maskb[:], in_=mask_d[:])
            nc.sync.dma_start(out=ones128[:], in_=ones_d[:])
            make_identity(nc, ident[:])
            for dq in range(GROUP):
                nc.sync.dma_start(
                    out=wq_s[:, ts(dq, DC), :], in_=wq_d[:, ts(dq, DC), :]
                )
            nc.sync.dma_start(out=wo_s[:], in_=wo_d[:])

            # ---- job bodies ----

            def proj_kv(tt, xa, xb):
                b = tt // 4

                def xsrc(Dc):
                    return (xa if Dc < HDC else xb)[:, Dc % HDC, :]

                ps = gps.tile([128, TOK], f32, tag="gemm", name="psk")
                for Dc in range(DC):
                    nc.tensor.matmul(
                        ps[:], lhsT=wk_s[:, Dc, :], rhs=xsrc(Dc),
                        start=(Dc == 0), stop=(Dc == DC - 1),
                    )
                nc.scalar.activation(
                    out=kT[:, ts(tt, TOK)], in_=ps[:], func=AF.Identity,
                    bias=bk_s[:, 0:1],
                )
                ps = gps.tile([128, TOK], f32, tag="gemm", name="psv")
                for Dc in range(DC):
                    nc.tensor.matmul(
                        ps[:], lhsT=wv_s[:, Dc, :], rhs=xsrc(Dc),
                        start=(Dc == 0), stop=(Dc == DC - 1),
                    )
                vT = mis.tile([128, TOK], bf16, tag="vt")
                nc.scalar.activation(
                    out=vT[:], in_=ps[:], func=AF.Identity, bias=bv_s[:, 0:1]
                )
                for j in range(4):
                    pt = gps.tile([128, 128], bf16, tag="gemm", name="pt")
                    nc.tensor.transpose(pt[:], vT[:, ts(j, 128)], ident[:])
                    nc.vector.tensor_copy(
                        out=vtm[:, b * KTB + (tt % 4) * 4 + j, :], in_=pt[:]
                    )

            def proj_q(tt, xa, xb, qt, dq):
                def xsrc(Dc):
                    return (xa if Dc < HDC else xb)[:, Dc % HDC, :]

                ps = gps.tile([128, TOK], f32, tag="gemm", name="psq")
                for Dc in range(DC):
                    nc.tensor.matmul(
                        ps[:], lhsT=wq_s[:, dq * DC + Dc, :], rhs=xsrc(Dc),
                        start=(Dc == 0), stop=(Dc == DC - 1),
                    )
                nc.scalar.activation(
                    out=qt[:, dq, :], in_=ps[:], func=AF.Identity,
                    bias=bq_s[:, dq : dq + 1],
                )

            def attn_head(tt, qt, yt, h, filler):
                qi, b = tt % 4, tt // 4
                njt = 4 * (qi + 1)
                yps = ypp.tile([128, TOK], f32, tag="yps")
                lps = lpp.tile([128, TOK], f32, tag="lps")
                for jp in range(njt // 2):
                    p16 = ppl.tile([128, 2, TOK], bf16, tag="p16", bufs=5)
                    for jj in range(2):
                        j = jp * 2 + jj
                        r = j - 4 * qi
                        st = stp.tile([128, TOK], f32, tag="st")
                        lo = 0 if r <= 0 else r * 128
                        nc.tensor.matmul(
                            st[:, lo:TOK],
                            lhsT=kT[:, ds(b * T + j * 128, 128)],
                            rhs=qt[:, h, lo:TOK],
                            start=True, stop=True,
                        )
                        if r >= 0:
                            nc.vector.tensor_add(
                                out=st[:], in0=st[:], in1=maskb[:, r, :]
                            )
                        nc.scalar.activation(
                            out=p16[:, jj, :], in_=st[:], func=AF.Exp,
                        )
                    for jj in range(2):
                        j = jp * 2 + jj
                        nc.tensor.matmul(
                            lps[:], lhsT=ones128[:], rhs=p16[:, jj, :],
                            start=(j == 0), stop=(j == njt - 1),
                        )
                        nc.tensor.matmul(
                            yps[:], lhsT=vtm[:, b * KTB + j, :],
                            rhs=p16[:, jj, :],
                            start=(j == 0), stop=(j == njt - 1),
                        )
                    filler()
                inv = mis.tile([128, TOK], f32, tag="inv", bufs=1)
                nc.vector.reciprocal_approx_fast(out=inv[:], in_=lps[:])
                nc.vector.tensor_mul(out=yt[:, h, :], in0=yps[:], in1=inv[:])

            def oproj_chunk(tt, yt, Do):
                ps = gps.tile([128, TOK], f32, tag="gemm", name="pso")
                for c in range(GROUP):
                    nc.tensor.matmul(
                        ps[:], lhsT=wo_s[:, c * DC + Do, :], rhs=yt[:, c, :],
                        start=(c == 0), stop=(c == GROUP - 1),
                    )
                so = mis.tile([128, TOK], f16, tag="so", bufs=3)
                nc.vector.tensor_copy(out=so[:], in_=ps[:])
                nc.sync.dma_start(out=out_d[:, Do, ts(tt, TOK)], in_=so[:])

            # ---- woven schedule ----
            # tt order: batches interleaved, causality respected.
            tt_order = [0, 4, 1, 5, 2, 6, 3, 7]
            heads = deque()  # pending attention-head jobs
            fillq = deque()  # pending oproj Do-chunk jobs (PE filler work)

            def filler():
                if fillq:
                    fillq.popleft()()

            def pump(n):
                for _ in range(n):
                    if heads:
                        heads.popleft()()
                    elif fillq:
                        fillq.popleft()()
                    else:
                        return

            def head_job(tt, h):
                attn_head(tt, qts[tt], yts[tt], h, filler)
                if h == GROUP - 1:
                    # all heads of tt emitted: its out-proj chunks are now
                    # safe to emit (yt fully written in program order)
                    for Do in range(DC):
                        fillq.append(
                            lambda tt=tt, Do=Do: oproj_chunk(tt, yts[tt], Do)
                        )

            xabs = {0: xab0}
            qts, yts = {}, {}
            for s, tt in enumerate(tt_order):
                if tt not in xabs:
                    xabs[tt] = dma_x(tt)
                xa, xb = xabs[tt]
                nxt = tt_order[s + 1] if s + 1 < len(tt_order) else None
                proj_kv(tt, xa, xb)
                if nxt is not None:
                    xabs[nxt] = dma_x(nxt)
                qt = qyp.tile([128, GROUP, TOK], bf16, tag="qt", bufs=2, name=f"qt{tt}")
                yt = qyp.tile([128, GROUP, TOK], bf16, tag="yt", bufs=3, name=f"yt{tt}")
                qts[tt], yts[tt] = qt, yt
                for dq in range(GROUP):
                    proj_q(tt, xa, xb, qt, dq)
                    pump(1)  # one head job
                    for _ in range(3):
                        filler()  # a few oproj chunks
                prev = tt
                for h in range(GROUP):
                    heads.append(lambda tt=prev, h=h: head_job(tt, h))
                for _ in range(4):
                    filler()
            # drain
            while heads or fillq:
                pump(1)
                for _ in range(3):
                    filler()

    if not nc.is_finalized():
        nc.finalize()
    return nc


def _prep_inputs(hidden_states, Wq, bq, Wk, bk, Wv, bv, Wo, bo):
    scale = 1.0 / math.sqrt(d)

    x_flat = np.asarray(hidden_states, dtype=np.float32).reshape(NT, D)
    xt = np.ascontiguousarray(
        x_flat.reshape(NT, DC, 128).transpose(2, 1, 0)
    ).astype(BF16)

    jj = np.arange(128)[:, None, None]
    rr = np.arange(4)[None, :, None] * 128
    ii = np.arange(TOK)[None, None, :]
    masks = np.where(jj + rr > ii, np.float32(-1e9), np.float32(0.0)).astype(BF16)
    masks = np.ascontiguousarray(masks)

    ones128 = np.ones((128, 128), dtype=BF16)

    in_maps = []
    for g in range(NC_):
        Wq_g = np.asarray(Wq[g * 512 : (g + 1) * 512, :], dtype=np.float32) * scale
        bq_g = np.asarray(bq[g * 512 : (g + 1) * 512], dtype=np.float32) * scale
        Wk_g = np.asarray(Wk[g * 128 : (g + 1) * 128, :], dtype=np.float32)
        bk_g = np.asarray(bk[g * 128 : (g + 1) * 128], dtype=np.float32)
        Wv_g = np.asarray(Wv[g * 128 : (g + 1) * 128], dtype=np.float32)
        bv_g = np.asarray(bv[g * 128 : (g + 1) * 128], dtype=np.float32)
        Wo_g = np.asarray(Wo[:, g * 512 : (g + 1) * 512], dtype=np.float32)

        wq_host = np.ascontiguousarray(
            Wq_g.reshape(GROUP, 128, DC, 128).transpose(3, 0, 2, 1).reshape(
                128, GROUP * DC, 128
            )
        ).astype(BF16)
        wk_host = np.ascontiguousarray(
            Wk_g.reshape(128, DC, 128).transpose(2, 1, 0)
        ).astype(BF16)
        wv_host = np.ascontiguousarray(
            Wv_g.reshape(128, DC, 128).transpose(2, 1, 0)
        ).astype(BF16)
        wo_host = np.ascontiguousarray(
            Wo_g.reshape(DC, 128, GROUP, 128).transpose(3, 2, 0, 1).reshape(
                128, GROUP * DC, 128
            )
        ).astype(BF16)

        in_maps.append(
            {
                "xt": xt,
                "wq": wq_host,
                "wk": wk_host,
                "wv": wv_host,
                "wo": wo_host,
                "bq": np.ascontiguousarray(bq_g.reshape(GROUP, 128).T),
                "bk": bk_g.reshape(128, 1).copy(),
                "bv": bv_g.reshape(128, 1).copy(),
                "masks": masks,
                "ones128": ones128,
            }
        )
    return in_maps


def kernel(
    hidden_states, Wq, bq, Wk, bk, Wv, bv, Wo, bo, _trace=False, _result_box=None
):
    from concourse.bass_utils import run_bass_kernel_spmd

    if "nc" not in _program_cache:
        _program_cache["nc"] = _build_program()
    nc = _program_cache["nc"]

    in_maps = _prep_inputs(hidden_states, Wq, bq, Wk, bk, Wv, bv, Wo, bo)
    res = run_bass_kernel_spmd(
        nc, in_maps, core_ids=list(range(NC_)), trace=_trace
    )
    if _result_box is not None:
        _result_box.append(res)

    acc = np.zeros((128, DC, NT), dtype=np.float32)
    for r in res.results:
        acc += r["out"].astype(np.float32)
    outT = acc.transpose(1, 0, 2).reshape(D, NT)
    out = outT.T + np.asarray(bo, dtype=np.float32)[None, :]
    return np.ascontiguousarray(out.reshape(B, T, D), dtype=np.float32)
